# revision 4
# baseline (speedup 1.0000x reference)
"""GAT-style graph-attention kernel for Trainium2, sharded over 8 NeuronCores.

Math (reference):
  h = x*conv_w + conv_b                       [N, D]
  Wh1 = h @ a1.T ; Wh2 = h @ a2.T             [N, H]
  e[k,i,j] = elu(Wh1[i,k] + Wh2[j,k])
  att = softmax_j(where(adj>0, e, -9e15))
  out = elu(0.5*mean_k(att@h) + 0.5*h); out /= max(||out||_2, 1e-12); out += bias

Key identities used on device:
  q := exp(z) = exp(w1_i)*exp(w2_j)  (outer product of tiny exp vectors)
  exp(elu(z)) = min(exp(q-1), max(q, 1))   (exact)
  p = mask*min(exp(q-1), max(q,1)) = min(mask*max(q,1), exp(q-1))
      (mask folds into the linear branch: u=exp(q-1)>0, so min(0,u)=0)
  softmax denominator via a ones-column appended to x in the att@x matmul;
  att@h = w*(att@x) + b (rows of att sum to 1), so conv w/b fold into the
  epilogue and the matmul rhs is raw bf16 x straight from DMA.

Main loop is 3 elementwise passes + 1 ACT pass per score tile, one engine
each (the baseline needed 5 passes + on-device mask/dtype converts):
  q  = qv1 (*) qwh          TS  mult   DVE (4x mode; 8 sections/tile)
  u  = exp(q - 1)           ACT Exp
  gm = mask * max(q, 1)     STT max,mult  Pool (fused)
  p  = min(gm, u)           TT  min    DVE (2x mode)
then p^T @ [x|1] accumulates over j in PSUM (bf16 matmuls).

Sharding: each core owns a 512-row block of the output for ALL 4 heads
(row-parallel; no collectives). Scores are built transposed (j on
partitions); the host passes the transposed bf16 adjacency mask per core.
"""
import sys

if "/opt/trn_rl_repo" not in sys.path:
    sys.path.insert(0, "/opt/trn_rl_repo")

import numpy as np
import ml_dtypes
from contextlib import ExitStack

import concourse.bass as bass
import concourse.tile as tile
from concourse import bacc, mybir

N, D, H = 4096, 256, 4
NCORES = 8
R = N // NCORES          # 512 rows per core
JT = N // 128            # 32 j-tiles
IC = R // 128            # 4 i-chunks per core
SB = 4                   # j-tiles per superblock
NSB = JT // SB           # 8 superblocks
HP = 2                   # heads per head-pair sweep
WID = HP * SB * R        # free width of a score tile (2*4*512 = 4096)
DA = D + 1               # x plus ones column

FP32 = mybir.dt.float32
BF16 = mybir.dt.bfloat16
AF = mybir.ActivationFunctionType
ALU = mybir.AluOpType


def _build_program(w_conv: float, b_conv: float):
    nc = bacc.Bacc("TRN2", target_bir_lowering=False, debug=False,
                   num_devices=NCORES)

    xb_d = nc.dram_tensor("xb", [N, D], BF16, kind="ExternalInput")
    xIb_d = nc.dram_tensor("xIb", [R, D], BF16, kind="ExternalInput")
    xTb_d = nc.dram_tensor("xTb", [D, N], BF16, kind="ExternalInput")
    xTIb_d = nc.dram_tensor("xTIb", [D, R], BF16, kind="ExternalInput")
    a8_d = nc.dram_tensor("a8", [D, 2 * H], BF16, kind="ExternalInput")
    maskT_d = nc.dram_tensor("maskT", [N, R], BF16, kind="ExternalInput")
    bias_d = nc.dram_tensor("bias", [1, D], FP32, kind="ExternalInput")
    out_d = nc.dram_tensor("out", [R, D], FP32, kind="ExternalOutput")

    with tile.TileContext(nc) as tc, ExitStack() as ctx:
        # ---------------- main-loop pools first (stable SBUF placement) ----
        qp = ctx.enter_context(tc.tile_pool(name="q", bufs=3))
        up = ctx.enter_context(tc.tile_pool(name="u", bufs=2))
        gp = ctx.enter_context(tc.tile_pool(name="gm", bufs=2))
        pp = ctx.enter_context(tc.tile_pool(name="p", bufs=3))
        ep = ctx.enter_context(tc.tile_pool(name="ep", bufs=4))

        per = ctx.enter_context(tc.tile_pool(name="per", bufs=1))
        # transposed {0,1} mask, bf16, layout (j-tile, i): one flat tile
        maskA = per.tile([128, JT * R], BF16, tag="maskA")
        # [x | 1] rhs tiles, one flat tile, per-j-tile stride DA
        haug = per.tile([128, JT * DA], BF16, tag="haug")
        qwh = [per.tile([128, 4 * 2 * H], FP32, tag=f"qwh{g}", name=f"qwh{g}")
               for g in range(JT // 4)]
        qv1bc = per.tile([128, H * R], BF16, tag="qv1bc")
        h_I = per.tile([128, IC * D], FP32, tag="h_I")   # 0.5*h + 0.5*b
        accp = [per.tile([128, D], FP32, tag=f"accp{icc}", name=f"accp{icc}")
                for icc in range(IC)]
        bias_bc = per.tile([128, D], FP32, tag="bias_bc")
        neg1 = per.tile([128, 1], FP32, tag="neg1")
        nc.gpsimd.memset(neg1[:], -1.0)

        # ---------------- setup ----------------
        stg = ctx.enter_context(tc.tile_pool(name="stage", bufs=4))
        with tc.tile_pool(name="setup", bufs=1) as sp, \
             tc.tile_pool(name="pwI", bufs=1, space="PSUM") as pwI_pool, \
             tc.tile_pool(name="pw", bufs=4, space="PSUM") as pw_pool:
            # qv1 path first (small, unblocks the main loop quickly).
            # h = x*w+b is folded into the Wh matmuls: Wh = w*(X@a8) + b*S,
            # S = colsum(a8); the b*S terms for BOTH halves ride qv1's exp
            # bias (q = e^{w*PW1} * e^{w*PW2} * e^{b*(S_k+S_{H+k})}).
            a8t = []
            for dc in range(2):
                a8c = sp.tile([128, 2 * H], BF16, tag=f"a8{dc}",
                              name=f"a8{dc}")
                nc.sync.dma_start(a8c[:], a8_d[dc * 128:(dc + 1) * 128, :])
                a8t.append(a8c)
            ones_col = sp.tile([128, 1], BF16, tag="ones_col")
            nc.gpsimd.memset(ones_col[:], 1.0)
            pS = pwI_pool.tile([2 * H, 1], FP32, tag="pS")
            for dc in range(2):
                nc.tensor.matmul(pS[:], a8t[dc][:], ones_col[:],
                                 start=(dc == 0), stop=(dc == 1))
            S12 = sp.tile([2 * H, 1], FP32, tag="S12")
            nc.vector.tensor_copy(S12[:], pS[:])
            Shi = sp.tile([H, 1], FP32, tag="Shi")
            nc.sync.dma_start(Shi[:], S12[H:2 * H, :])
            qbias = sp.tile([H, 1], FP32, tag="qbias")
            nc.vector.tensor_add(qbias[:], S12[0:H, :], Shi[:])
            nc.vector.tensor_scalar(qbias[:], qbias[:], b_conv, None,
                                    op0=ALU.mult)
            xtiI = []
            for dc in range(2):
                xti = sp.tile([128, R], BF16, tag=f"xTI{dc}", name=f"xTI{dc}")
                nc.sync.dma_start(xti[:], xTIb_d[dc * 128:(dc + 1) * 128, :])
                xtiI.append(xti)
            pwI = pwI_pool.tile([2 * H, R], FP32, tag="pwI")
            for dc in range(2):
                nc.tensor.matmul(pwI[:], a8t[dc][:], xtiI[dc][:],
                                 start=(dc == 0), stop=(dc == 1))
            qwhTI = sp.tile([2 * H, R], BF16, tag="qwhTI")
            nc.scalar.activation(qwhTI[0:H, :], pwI[0:H, :], AF.Exp,
                                 bias=qbias[:], scale=w_conv)
            qv1row = sp.tile([1, H * R], BF16, tag="qv1row")
            nc.sync.dma_start(
                qv1row[:].rearrange("o (k i) -> o k i", k=H), qwhTI[0:H, :])
            nc.gpsimd.partition_broadcast(qv1bc[:], qv1row[:])

            # Wh2[j,:] for all j from raw x^T chunks (exp applies w_conv)
            for grp in range(JT // 4):
                hTg = []
                for dc in range(2):
                    xtc = stg.tile([128, 512], BF16, tag="xtc",
                                   name=f"xtc{grp}_{dc}")
                    nc.sync.dma_start(
                        xtc[:], xTb_d[dc * 128:(dc + 1) * 128,
                                      grp * 512:(grp + 1) * 512])
                    hTg.append(xtc)
                pw = pw_pool.tile([128, 4 * 2 * H], FP32, tag="pw",
                                  name=f"pw{grp}")
                for jb in range(grp * 4, grp * 4 + 4):
                    off = (jb % 4) * 128
                    col = (jb % 4) * 2 * H
                    for dc in range(2):
                        nc.tensor.matmul(
                            pw[:, col:col + 2 * H],
                            hTg[dc][:, off:off + 128], a8t[dc][:],
                            start=(dc == 0), stop=(dc == 1))
                nc.scalar.activation(qwh[grp][:], pw[:], AF.Exp, scale=w_conv)

            bias_row = sp.tile([1, D], FP32, tag="bias_row")
            nc.sync.dma_start(bias_row[:], bias_d[:, :])
            nc.gpsimd.partition_broadcast(bias_bc[:], bias_row[:])

            # h_I = 0.5*h + 0.5*b = 0.5*w*x + b on own rows
            for icc in range(IC):
                xi = stg.tile([128, D], BF16, tag="xistg", name=f"xi{icc}")
                nc.sync.dma_start(xi[:], xIb_d[icc * 128:(icc + 1) * 128, :])
                nc.scalar.activation(
                    h_I[:, icc * D:(icc + 1) * D], xi[:], AF.Copy,
                    bias=b_conv, scale=0.5 * w_conv)

        # ones columns of haug: one strided memset over all 32 j-tiles
        nc.gpsimd.memset(
            haug[:].rearrange("p (j d) -> p j d", j=JT)[:, :, D:D + 1], 1.0)

        # ---------------- main: score tiles + matmul ----------------
        # score tile free layout: (head-local, j-tile, i) = [HP, SB, R]
        with tc.tile_pool(name="pm", bufs=1, space="PSUM") as pmp:
            for hp in range(2):
                heads = (2 * hp, 2 * hp + 1)
                pm = [[pmp.tile([128, DA], FP32, tag=f"pm{hl}{icc}",
                                name=f"pm{hl}{icc}_{hp}")
                       for icc in range(IC)] for hl in range(HP)]
                for sb in range(NSB):
                    jb0 = SB * sb
                    if hp == 0:
                        # stream this superblock's mask + x rows (batched:
                        # SB j-tiles per transfer via 3-D access patterns)
                        nc.sync.dma_start(
                            maskA[:, jb0 * R:(jb0 + SB) * R]
                            .rearrange("p (j i) -> p j i", j=SB),
                            maskT_d[jb0 * 128:(jb0 + SB) * 128, :]
                            .rearrange("(j p) i -> p j i", j=SB))
                        nc.scalar.dma_start(
                            haug[:, jb0 * DA:(jb0 + SB) * DA]
                            .rearrange("p (j d) -> p j d", j=SB)[:, :, 0:D],
                            xb_d[jb0 * 128:(jb0 + SB) * 128, :]
                            .rearrange("(j p) d -> p j d", j=SB))
                    it = hp * NSB + sb
                    g = qp.tile([128, WID], BF16, tag="g")
                    u = up.tile([128, WID], BF16, tag="u")
                    p0 = gp.tile([128, WID], BF16, tag="p0")
                    p = pp.tile([128, WID], BF16, tag="p")
                    if it % 2 == 0:
                        # structure A: raw-q sections (DVE 4x), wide exp
                        # (ACT), then clamp (DVE 4x). DVE-heavy, ACT-light.
                        q = qp.tile([128, WID], BF16, tag="qA")
                        for hl in range(HP):
                            for jl in range(SB):
                                sec = (hl * SB + jl) * R
                                jb = jb0 + jl
                                qc = (jb % 4) * 2 * H + H + heads[hl]
                                nc.vector.tensor_scalar(
                                    q[:, sec:sec + R],
                                    qv1bc[:, heads[hl] * R:(heads[hl] + 1) * R],
                                    qwh[jb // 4][:, qc:qc + 1],
                                    None, op0=ALU.mult)
                        nc.scalar.activation(u[:], q[:], AF.Exp, bias=neg1[:])
                        nc.vector.tensor_scalar(g[:], q[:], 1.0, None,
                                                op0=ALU.max)
                    else:
                        # structure B: fused clamp sections g=max(q*qwh,1)
                        # (DVE 4x) + per-section ACT exp with qwh as the
                        # activation scale (no raw q). ACT-heavy, DVE-light.
                        for hl in range(HP):
                            qv1s = qv1bc[:, heads[hl] * R:(heads[hl] + 1) * R]
                            for jl in range(SB):
                                sec = (hl * SB + jl) * R
                                jb = jb0 + jl
                                qc = (jb % 4) * 2 * H + H + heads[hl]
                                qwc = qwh[jb // 4][:, qc:qc + 1]
                                nc.vector.tensor_scalar(
                                    g[:, sec:sec + R], qv1s, qwc, 1.0,
                                    op0=ALU.mult, op1=ALU.max)
                                nc.scalar.activation(
                                    u[:, sec:sec + R], qv1s, AF.Exp,
                                    bias=neg1[:], scale=qwc)
                    # p0 = min(g, u)  [DVE 2x];  p = p0 * mask  [Pool]
                    nc.vector.tensor_tensor(p0[:], g[:], u[:], op=ALU.min)
                    p03 = p0[:].rearrange("p (h ji) -> p h ji", h=HP)
                    p3 = p[:].rearrange("p (h ji) -> p h ji", h=HP)
                    mrep = (maskA[:, jb0 * R:(jb0 + SB) * R].unsqueeze(1)
                            .to_broadcast([128, HP, SB * R]))
                    nc.gpsimd.tensor_tensor(p3, p03, mrep, op=ALU.mult)
                    # accumulate p^T @ [x|1] over j into PSUM
                    for jl in range(SB):
                        rhs = haug[:, (jb0 + jl) * DA:(jb0 + jl + 1) * DA]
                        for hl in range(HP):
                            for icc in range(IC):
                                sec = (hl * SB + jl) * R + icc * 128
                                nc.tensor.matmul(
                                    pm[hl][icc][:], p[:, sec:sec + 128], rhs,
                                    start=(sb == 0 and jl == 0),
                                    stop=(sb == NSB - 1 and jl == SB - 1))
                # fold this head-pair into accp: accp += pm[:, :D] / s
                for hl in range(HP):
                    for icc in range(IC):
                        rcp = ep.tile([128, 1], FP32, tag="rcp")
                        nc.vector.reciprocal(rcp[:], pm[hl][icc][:, D:D + 1])
                        acs = accp[icc][:]
                        if hp == 0 and hl == 0:
                            nc.vector.tensor_scalar(
                                acs, pm[hl][icc][:, :D], rcp[:], None,
                                op0=ALU.mult)
                        else:
                            nc.vector.scalar_tensor_tensor(
                                acs, pm[hl][icc][:, :D], rcp[:], acs,
                                op0=ALU.mult, op1=ALU.add)

            # ---------------- epilogue ----------------
            # Phase 1: everything except Sqrt (keeps the Exp ACT table).
            o_t = []
            for icc in range(IC):
                acs = accp[icc][:]
                t = ep.tile([128, D], FP32, tag="t", name=f"t{icc}")
                # t = 0.125*w*accp + (0.5*h + 0.5*b)  (accp holds att@x)
                nc.vector.scalar_tensor_tensor(
                    t[:], acs, 0.125 * w_conv, h_I[:, icc * D:(icc + 1) * D],
                    op0=ALU.mult, op1=ALU.add)
                # elu(t) = relu(t) + min(exp(t), 1) - 1
                eq = ep.tile([128, D], FP32, tag="eq")
                nc.scalar.activation(eq[:], t[:], AF.Exp)
                o1 = ep.tile([128, D], FP32, tag="o1")
                nc.vector.tensor_scalar(o1[:], eq[:], 1.0, -1.0,
                                        op0=ALU.min, op1=ALU.add)
                o = ep.tile([128, D], FP32, tag="o", name=f"o{icc}")
                nc.vector.scalar_tensor_tensor(o[:], t[:], 0.0, o1[:],
                                               op0=ALU.max, op1=ALU.add)
                o_t.append(o)
            # Phase 2: batched Sqrt (single table switch), then normalize.
            for icc in range(IC):
                o = o_t[icc]
                sq = ep.tile([128, D], FP32, tag="sq")
                ss = ep.tile([128, 1], FP32, tag="ss")
                nc.vector.tensor_mul(sq[:], o[:], o[:])
                nc.vector.tensor_reduce(ss[:], sq[:],
                                        axis=mybir.AxisListType.X, op=ALU.add)
                nrm = ep.tile([128, 1], FP32, tag="nrm")
                nc.scalar.activation(nrm[:], ss[:], AF.Sqrt)
                nrm2 = ep.tile([128, 1], FP32, tag="nrm2")
                nc.vector.tensor_scalar(nrm2[:], nrm[:], 1e-12, None,
                                        op0=ALU.max)
                rcpn = ep.tile([128, 1], FP32, tag="rcpn")
                nc.vector.reciprocal(rcpn[:], nrm2[:])
                outv = ep.tile([128, D], FP32, tag="outv")
                nc.vector.scalar_tensor_tensor(
                    outv[:], o[:], rcpn[:], bias_bc[:],
                    op0=ALU.mult, op1=ALU.add)
                nc.sync.dma_start(out_d[icc * 128:(icc + 1) * 128, :], outv[:])

    nc.finalize()
    return nc


_PROGRAM_CACHE = {}


def _get_program(w_conv: float, b_conv: float):
    key = (w_conv, b_conv)
    if key not in _PROGRAM_CACHE:
        _PROGRAM_CACHE[key] = _build_program(w_conv, b_conv)
    return _PROGRAM_CACHE[key]


def kernel(x, adj, conv_w, conv_b, a, bias, _want_results=False, _trace=False,
           **_ignored):
    from concourse.bass_utils import run_bass_kernel_spmd

    x = np.asarray(x, dtype=np.float32)
    adj = np.ascontiguousarray(np.asarray(adj, dtype=np.int32))
    a = np.asarray(a, dtype=np.float32)
    bias = np.asarray(bias, dtype=np.float32)
    w_conv = float(np.asarray(conv_w).reshape(-1)[0])
    b_conv = float(np.asarray(conv_b).reshape(-1)[0])

    xn = np.ascontiguousarray(x.reshape(N, D))
    xb = np.ascontiguousarray(xn.astype(ml_dtypes.bfloat16))
    xTb = np.ascontiguousarray(xn.T.astype(ml_dtypes.bfloat16))
    a1 = a[:, :D, 0]
    a2 = a[:, D:, 0]
    a8 = np.ascontiguousarray(
        np.concatenate([a1, a2], axis=0).T.astype(ml_dtypes.bfloat16))
    bias_row = np.ascontiguousarray(bias.reshape(1, D))
    maskb = adj.astype(ml_dtypes.bfloat16)

    nc = _get_program(w_conv, b_conv)

    in_maps = []
    for c in range(NCORES):
        rows = slice(c * R, (c + 1) * R)
        in_maps.append({
            "xb": xb,
            "xIb": np.ascontiguousarray(xb[rows]),
            "xTb": xTb,
            "xTIb": np.ascontiguousarray(xTb[:, rows]),
            "a8": a8,
            "maskT": np.ascontiguousarray(maskb[rows].T),
            "bias": bias_row,
        })
    res = run_bass_kernel_spmd(nc, in_maps, core_ids=list(range(NCORES)),
                               trace=_trace)
    out = np.concatenate([res.results[c]["out"] for c in range(NCORES)], axis=0)
    if _want_results:
        return out, res
    return out


# revision 5
# speedup vs baseline: 1.4402x; 1.4402x over previous
"""GAT-style graph-attention kernel for Trainium2, sharded over 8 NeuronCores.

Math (reference):
  h = x*conv_w + conv_b                       [N, D]
  Wh1 = h @ a1.T ; Wh2 = h @ a2.T             [N, H]
  e[k,i,j] = elu(Wh1[i,k] + Wh2[j,k])
  att = softmax_j(where(adj>0, e, -9e15))
  out = elu(0.5*mean_k(att@h) + 0.5*h); out /= max(||out||_2, 1e-12); out += bias

Key identities used on device:
  q := exp(z) = exp(w1_i)*exp(w2_j)  (outer product of tiny exp vectors)
  exp(elu(z)) = min(exp(q-1), max(q, 1))   (exact)
  p = mask*min(exp(q-1), max(q,1)) = min(mask*max(q,1), exp(q-1))
      (mask folds into the linear branch: u=exp(q-1)>0, so min(0,u)=0)
  softmax denominator via a ones-column appended to x in the att@x matmul;
  att@h = w*(att@x) + b (rows of att sum to 1), so conv w/b fold into the
  epilogue and the matmul rhs is raw bf16 x straight from DMA.

Main loop is 3 elementwise passes + 1 ACT pass per score tile, one engine
each (the baseline needed 5 passes + on-device mask/dtype converts):
  q  = qv1 (*) qwh          TS  mult   DVE (4x mode; 8 sections/tile)
  u  = exp(q - 1)           ACT Exp
  gm = mask * max(q, 1)     STT max,mult  Pool (fused)
  p  = min(gm, u)           TT  min    DVE (2x mode)
then p^T @ [x|1] accumulates over j in PSUM (bf16 matmuls).

Sharding: each core owns a 512-row block of the output for ALL 4 heads
(row-parallel; no collectives). Scores are built transposed (j on
partitions); the host passes the transposed bf16 adjacency mask per core.
"""
import sys

if "/opt/trn_rl_repo" not in sys.path:
    sys.path.insert(0, "/opt/trn_rl_repo")

import numpy as np
import ml_dtypes
from contextlib import ExitStack

import concourse.bass as bass
import concourse.tile as tile
from concourse import bacc, mybir

N, D, H = 4096, 256, 4
NCORES = 8
R = N // NCORES          # 512 rows per core
JT = N // 128            # 32 j-tiles
IC = R // 128            # 4 i-chunks per core
SB = 4                   # j-tiles per superblock
NSB = JT // SB           # 8 superblocks
HP = 2                   # heads per head-pair sweep
WID = HP * SB * R        # free width of a score tile (2*4*512 = 4096)
DA = D + 1               # x plus ones column

FP32 = mybir.dt.float32
BF16 = mybir.dt.bfloat16
AF = mybir.ActivationFunctionType
ALU = mybir.AluOpType


def _build_program(w_conv: float, b_conv: float):
    nc = bacc.Bacc("TRN2", target_bir_lowering=False, debug=False,
                   num_devices=NCORES)

    xb_d = nc.dram_tensor("xb", [N, D], BF16, kind="ExternalInput")
    xIb_d = nc.dram_tensor("xIb", [R, D], BF16, kind="ExternalInput")
    xTb_d = nc.dram_tensor("xTb", [D, N], BF16, kind="ExternalInput")
    xTIb_d = nc.dram_tensor("xTIb", [D, R], BF16, kind="ExternalInput")
    a8_d = nc.dram_tensor("a8", [D, 2 * H], BF16, kind="ExternalInput")
    maskT_d = nc.dram_tensor("maskT", [N, R], BF16, kind="ExternalInput")
    bias_d = nc.dram_tensor("bias", [1, D], FP32, kind="ExternalInput")
    out_d = nc.dram_tensor("out", [R, D], FP32, kind="ExternalOutput")

    with tile.TileContext(nc) as tc, ExitStack() as ctx:
        # ---------------- main-loop pools first (stable SBUF placement) ----
        qp = ctx.enter_context(tc.tile_pool(name="q", bufs=3))
        up = ctx.enter_context(tc.tile_pool(name="u", bufs=2))
        gp = ctx.enter_context(tc.tile_pool(name="gm", bufs=2))
        pp = ctx.enter_context(tc.tile_pool(name="p", bufs=3))
        ep = ctx.enter_context(tc.tile_pool(name="ep", bufs=4))

        per = ctx.enter_context(tc.tile_pool(name="per", bufs=1))
        # transposed {0,1} mask, bf16, layout (j-tile, i): one flat tile
        maskA = per.tile([128, JT * R], BF16, tag="maskA")
        # [x | 1] rhs tiles, one flat tile, per-j-tile stride DA
        haug = per.tile([128, JT * DA], BF16, tag="haug")
        qwh = [per.tile([128, 4 * 2 * H], FP32, tag=f"qwh{g}", name=f"qwh{g}")
               for g in range(JT // 4)]
        qv1bc = per.tile([128, H * R], BF16, tag="qv1bc")
        h_I = per.tile([128, IC * D], FP32, tag="h_I")   # 0.5*h + 0.5*b
        accp = [per.tile([128, D], FP32, tag=f"accp{icc}", name=f"accp{icc}")
                for icc in range(IC)]
        bias_bc = per.tile([128, D], FP32, tag="bias_bc")
        neg1 = per.tile([128, 1], FP32, tag="neg1")
        nc.gpsimd.memset(neg1[:], -1.0)

        # ---------------- setup ----------------
        stg = ctx.enter_context(tc.tile_pool(name="stage", bufs=4))
        with tc.tile_pool(name="setup", bufs=1) as sp, \
             tc.tile_pool(name="pwI", bufs=1, space="PSUM") as pwI_pool, \
             tc.tile_pool(name="pw", bufs=4, space="PSUM") as pw_pool:
            # qv1 path first (small, unblocks the main loop quickly).
            # h = x*w+b is folded into the Wh matmuls: Wh = w*(X@a8) + b*S,
            # S = colsum(a8); the b*S terms for BOTH halves ride qv1's exp
            # bias (q = e^{w*PW1} * e^{w*PW2} * e^{b*(S_k+S_{H+k})}).
            a8t = []
            for dc in range(2):
                a8c = sp.tile([128, 2 * H], BF16, tag=f"a8{dc}",
                              name=f"a8{dc}")
                nc.sync.dma_start(a8c[:], a8_d[dc * 128:(dc + 1) * 128, :])
                a8t.append(a8c)
            ones_col = sp.tile([128, 1], BF16, tag="ones_col")
            nc.gpsimd.memset(ones_col[:], 1.0)
            pS = pwI_pool.tile([2 * H, 1], FP32, tag="pS")
            for dc in range(2):
                nc.tensor.matmul(pS[:], a8t[dc][:], ones_col[:],
                                 start=(dc == 0), stop=(dc == 1))
            S12 = sp.tile([2 * H, 1], FP32, tag="S12")
            nc.vector.tensor_copy(S12[:], pS[:])
            Shi = sp.tile([H, 1], FP32, tag="Shi")
            nc.sync.dma_start(Shi[:], S12[H:2 * H, :])
            qbias = sp.tile([H, 1], FP32, tag="qbias")
            nc.vector.tensor_add(qbias[:], S12[0:H, :], Shi[:])
            nc.vector.tensor_scalar(qbias[:], qbias[:], b_conv, None,
                                    op0=ALU.mult)
            xtiI = []
            for dc in range(2):
                xti = sp.tile([128, R], BF16, tag=f"xTI{dc}", name=f"xTI{dc}")
                nc.sync.dma_start(xti[:], xTIb_d[dc * 128:(dc + 1) * 128, :])
                xtiI.append(xti)
            pwI = pwI_pool.tile([2 * H, R], FP32, tag="pwI")
            for dc in range(2):
                nc.tensor.matmul(pwI[:], a8t[dc][:], xtiI[dc][:],
                                 start=(dc == 0), stop=(dc == 1))
            qwhTI = sp.tile([2 * H, R], BF16, tag="qwhTI")
            nc.scalar.activation(qwhTI[0:H, :], pwI[0:H, :], AF.Exp,
                                 bias=qbias[:], scale=w_conv)
            qv1row = sp.tile([1, H * R], BF16, tag="qv1row")
            nc.sync.dma_start(
                qv1row[:].rearrange("o (k i) -> o k i", k=H), qwhTI[0:H, :])
            nc.gpsimd.partition_broadcast(qv1bc[:], qv1row[:])

            # Wh2[j,:] for all j from raw x^T chunks (exp applies w_conv)
            for grp in range(JT // 4):
                hTg = []
                for dc in range(2):
                    xtc = stg.tile([128, 512], BF16, tag="xtc",
                                   name=f"xtc{grp}_{dc}")
                    nc.sync.dma_start(
                        xtc[:], xTb_d[dc * 128:(dc + 1) * 128,
                                      grp * 512:(grp + 1) * 512])
                    hTg.append(xtc)
                pw = pw_pool.tile([128, 4 * 2 * H], FP32, tag="pw",
                                  name=f"pw{grp}")
                for jb in range(grp * 4, grp * 4 + 4):
                    off = (jb % 4) * 128
                    col = (jb % 4) * 2 * H
                    for dc in range(2):
                        nc.tensor.matmul(
                            pw[:, col:col + 2 * H],
                            hTg[dc][:, off:off + 128], a8t[dc][:],
                            start=(dc == 0), stop=(dc == 1))
                nc.scalar.activation(qwh[grp][:], pw[:], AF.Exp, scale=w_conv)

            bias_row = sp.tile([1, D], FP32, tag="bias_row")
            nc.sync.dma_start(bias_row[:], bias_d[:, :])
            nc.gpsimd.partition_broadcast(bias_bc[:], bias_row[:])

            # h_I = 0.5*h + 0.5*b = 0.5*w*x + b on own rows
            for icc in range(IC):
                xi = stg.tile([128, D], BF16, tag="xistg", name=f"xi{icc}")
                nc.sync.dma_start(xi[:], xIb_d[icc * 128:(icc + 1) * 128, :])
                nc.scalar.activation(
                    h_I[:, icc * D:(icc + 1) * D], xi[:], AF.Copy,
                    bias=b_conv, scale=0.5 * w_conv)

        # ones columns of haug: one strided memset over all 32 j-tiles
        nc.gpsimd.memset(
            haug[:].rearrange("p (j d) -> p j d", j=JT)[:, :, D:D + 1], 1.0)

        # ---------------- main: score tiles + matmul ----------------
        # score tile free layout: (head-local, j-tile, i) = [HP, SB, R]
        with tc.tile_pool(name="pm", bufs=1, space="PSUM") as pmp:
            for hp in range(2):
                heads = (2 * hp, 2 * hp + 1)
                pm = [[pmp.tile([128, DA], FP32, tag=f"pm{hl}{icc}",
                                name=f"pm{hl}{icc}_{hp}")
                       for icc in range(IC)] for hl in range(HP)]
                for sb in range(NSB):
                    jb0 = SB * sb
                    if hp == 0:
                        # stream this superblock's mask + x rows (batched:
                        # SB j-tiles per transfer via 3-D access patterns)
                        nc.sync.dma_start(
                            maskA[:, jb0 * R:(jb0 + SB) * R]
                            .rearrange("p (j i) -> p j i", j=SB),
                            maskT_d[jb0 * 128:(jb0 + SB) * 128, :]
                            .rearrange("(j p) i -> p j i", j=SB))
                        nc.scalar.dma_start(
                            haug[:, jb0 * DA:(jb0 + SB) * DA]
                            .rearrange("p (j d) -> p j d", j=SB)[:, :, 0:D],
                            xb_d[jb0 * 128:(jb0 + SB) * 128, :]
                            .rearrange("(j p) d -> p j d", j=SB))
                    it = hp * NSB + sb
                    g = qp.tile([128, WID], BF16, tag="g")
                    u = up.tile([128, WID], BF16, tag="u")
                    p0 = gp.tile([128, WID], BF16, tag="p0")
                    p = pp.tile([128, WID], BF16, tag="p")
                    if it % 8 == 0:
                        # structure A (2 of 16 iters): raw-q sections (DVE
                        # 4x), one wide exp (ACT), clamp (DVE 4x TS).
                        # DVE-heavy, ACT-light.
                        q = qp.tile([128, WID], BF16, tag="qA")
                        for hl in range(HP):
                            for jl in range(SB):
                                sec = (hl * SB + jl) * R
                                jb = jb0 + jl
                                qc = (jb % 4) * 2 * H + H + heads[hl]
                                nc.vector.tensor_scalar(
                                    q[:, sec:sec + R],
                                    qv1bc[:, heads[hl] * R:(heads[hl] + 1) * R],
                                    qwh[jb // 4][:, qc:qc + 1],
                                    None, op0=ALU.mult)
                        nc.scalar.activation(u[:], q[:], AF.Exp, bias=neg1[:])
                        nc.vector.tensor_scalar(g[:], q[:], 1.0, None,
                                                op0=ALU.max)
                    else:
                        # structure B: fused clamp sections g=max(q*qwh,1)
                        # (DVE 4x) + per-section ACT exp with qwh as the
                        # activation scale (raw q never materialized).
                        # ACT-heavy, DVE-light.
                        for hl in range(HP):
                            qv1s = qv1bc[:, heads[hl] * R:(heads[hl] + 1) * R]
                            for jl in range(SB):
                                sec = (hl * SB + jl) * R
                                jb = jb0 + jl
                                qc = (jb % 4) * 2 * H + H + heads[hl]
                                qwc = qwh[jb // 4][:, qc:qc + 1]
                                nc.vector.tensor_scalar(
                                    g[:, sec:sec + R], qv1s, qwc, 1.0,
                                    op0=ALU.mult, op1=ALU.max)
                                nc.scalar.activation(
                                    u[:, sec:sec + R], qv1s, AF.Exp,
                                    bias=neg1[:], scale=qwc)
                    # p0 = min(g, u)  [DVE 2x]
                    nc.vector.tensor_tensor(p0[:], g[:], u[:], op=ALU.min)
                    # p = p0 * mask: head hl=0 half on DVE (2x), hl=1 half
                    # on Pool — splits the TT load and lets the hl=0
                    # matmuls start while Pool masks hl=1.
                    mrep = (maskA[:, jb0 * R:(jb0 + SB) * R].unsqueeze(1)
                            .to_broadcast([128, 1, SB * R]))
                    HW = SB * R
                    nc.vector.tensor_tensor(
                        p[:, 0:HW].rearrange("p (o w) -> p o w", o=1),
                        p0[:, 0:HW].rearrange("p (o w) -> p o w", o=1),
                        mrep, op=ALU.mult)
                    nc.gpsimd.tensor_tensor(
                        p[:, HW:2 * HW].rearrange("p (o w) -> p o w", o=1),
                        p0[:, HW:2 * HW].rearrange("p (o w) -> p o w", o=1),
                        mrep, op=ALU.mult)
                    # accumulate p^T @ [x|1] over j into PSUM
                    for jl in range(SB):
                        rhs = haug[:, (jb0 + jl) * DA:(jb0 + jl + 1) * DA]
                        for hl in range(HP):
                            for icc in range(IC):
                                sec = (hl * SB + jl) * R + icc * 128
                                nc.tensor.matmul(
                                    pm[hl][icc][:], p[:, sec:sec + 128], rhs,
                                    start=(sb == 0 and jl == 0),
                                    stop=(sb == NSB - 1 and jl == SB - 1))
                # fold this head-pair into accp: accp += pm[:, :D] / s
                for hl in range(HP):
                    for icc in range(IC):
                        rcp = ep.tile([128, 1], FP32, tag="rcp")
                        nc.vector.reciprocal(rcp[:], pm[hl][icc][:, D:D + 1])
                        acs = accp[icc][:]
                        if hp == 0 and hl == 0:
                            nc.vector.tensor_scalar(
                                acs, pm[hl][icc][:, :D], rcp[:], None,
                                op0=ALU.mult)
                        else:
                            nc.vector.scalar_tensor_tensor(
                                acs, pm[hl][icc][:, :D], rcp[:], acs,
                                op0=ALU.mult, op1=ALU.add)

            # ---------------- epilogue ----------------
            # Phase 1: everything except Sqrt (keeps the Exp ACT table).
            o_t = []
            for icc in range(IC):
                acs = accp[icc][:]
                t = ep.tile([128, D], FP32, tag="t", name=f"t{icc}")
                # t = 0.125*w*accp + (0.5*h + 0.5*b)  (accp holds att@x)
                nc.vector.scalar_tensor_tensor(
                    t[:], acs, 0.125 * w_conv, h_I[:, icc * D:(icc + 1) * D],
                    op0=ALU.mult, op1=ALU.add)
                # elu(t) = relu(t) + min(exp(t), 1) - 1
                eq = ep.tile([128, D], FP32, tag="eq")
                nc.scalar.activation(eq[:], t[:], AF.Exp)
                o1 = ep.tile([128, D], FP32, tag="o1")
                nc.vector.tensor_scalar(o1[:], eq[:], 1.0, -1.0,
                                        op0=ALU.min, op1=ALU.add)
                o = ep.tile([128, D], FP32, tag="o", name=f"o{icc}")
                nc.vector.scalar_tensor_tensor(o[:], t[:], 0.0, o1[:],
                                               op0=ALU.max, op1=ALU.add)
                o_t.append(o)
            # Phase 2: batched Sqrt (single table switch), then normalize.
            for icc in range(IC):
                o = o_t[icc]
                sq = ep.tile([128, D], FP32, tag="sq")
                ss = ep.tile([128, 1], FP32, tag="ss")
                nc.vector.tensor_mul(sq[:], o[:], o[:])
                nc.vector.tensor_reduce(ss[:], sq[:],
                                        axis=mybir.AxisListType.X, op=ALU.add)
                nrm = ep.tile([128, 1], FP32, tag="nrm")
                nc.scalar.activation(nrm[:], ss[:], AF.Sqrt)
                nrm2 = ep.tile([128, 1], FP32, tag="nrm2")
                nc.vector.tensor_scalar(nrm2[:], nrm[:], 1e-12, None,
                                        op0=ALU.max)
                rcpn = ep.tile([128, 1], FP32, tag="rcpn")
                nc.vector.reciprocal(rcpn[:], nrm2[:])
                outv = ep.tile([128, D], FP32, tag="outv")
                nc.vector.scalar_tensor_tensor(
                    outv[:], o[:], rcpn[:], bias_bc[:],
                    op0=ALU.mult, op1=ALU.add)
                nc.sync.dma_start(out_d[icc * 128:(icc + 1) * 128, :], outv[:])

    nc.finalize()
    return nc


_PROGRAM_CACHE = {}


def _get_program(w_conv: float, b_conv: float):
    key = (w_conv, b_conv)
    if key not in _PROGRAM_CACHE:
        _PROGRAM_CACHE[key] = _build_program(w_conv, b_conv)
    return _PROGRAM_CACHE[key]


def kernel(x, adj, conv_w, conv_b, a, bias, _want_results=False, _trace=False,
           **_ignored):
    from concourse.bass_utils import run_bass_kernel_spmd

    x = np.asarray(x, dtype=np.float32)
    adj = np.ascontiguousarray(np.asarray(adj, dtype=np.int32))
    a = np.asarray(a, dtype=np.float32)
    bias = np.asarray(bias, dtype=np.float32)
    w_conv = float(np.asarray(conv_w).reshape(-1)[0])
    b_conv = float(np.asarray(conv_b).reshape(-1)[0])

    xn = np.ascontiguousarray(x.reshape(N, D))
    xb = np.ascontiguousarray(xn.astype(ml_dtypes.bfloat16))
    xTb = np.ascontiguousarray(xn.T.astype(ml_dtypes.bfloat16))
    a1 = a[:, :D, 0]
    a2 = a[:, D:, 0]
    a8 = np.ascontiguousarray(
        np.concatenate([a1, a2], axis=0).T.astype(ml_dtypes.bfloat16))
    bias_row = np.ascontiguousarray(bias.reshape(1, D))
    maskb = adj.astype(ml_dtypes.bfloat16)

    nc = _get_program(w_conv, b_conv)

    in_maps = []
    for c in range(NCORES):
        rows = slice(c * R, (c + 1) * R)
        in_maps.append({
            "xb": xb,
            "xIb": np.ascontiguousarray(xb[rows]),
            "xTb": xTb,
            "xTIb": np.ascontiguousarray(xTb[:, rows]),
            "a8": a8,
            "maskT": np.ascontiguousarray(maskb[rows].T),
            "bias": bias_row,
        })
    res = run_bass_kernel_spmd(nc, in_maps, core_ids=list(range(NCORES)),
                               trace=_trace)
    out = np.concatenate([res.results[c]["out"] for c in range(NCORES)], axis=0)
    if _want_results:
        return out, res
    return out
